# revision 24
# baseline (speedup 1.0000x reference)
"""Distributed GIN (3-layer) kernel for 8 TRN2 NeuronCores.

Sharding: nodes (and their incident in-edges) are partitioned across the 8
cores; each core keeps its node-feature shard resident in SBUF (transposed,
bf16).  Between layers the per-core shards are AllGathered into bf16
"gather tables" in DRAM — chunked 4 ways, with gather QUADRANT == AG CHUNK,
so each chunk's AllGather is issued as soon as its blocks are computed and
overlaps the remaining compute; the next layer's gathers per quadrant wait
only on their own chunk.  Neighbor aggregation is a dma_gather (256B/edge,
spread across 4 SWDGE queues so all four Q7 core-pairs emit descriptors in
parallel) + one-hot matmul segment-sum on the tensor engine.  MLP weights
are replicated (bf16).  Per-graph pooled sums are computed per-core on the
PE (host-precomputed one-hot pooling masks) and reduced on the host.
"""

import math

import ml_dtypes
import numpy as np

N_NODES = 100000
N_EDGES = 1600000
F = 128              # feature dim (= hidden dim)
L = 3                # GIN layers
G = 64               # graphs
NC = 8               # NeuronCores
PER = N_NODES // NC  # 12500 real nodes per core
BLK = 128
NBLK = (PER + BLK - 1) // BLK          # 98 node blocks per core
PERP = NBLK * BLK                      # 12544 padded rows per core
NQ = 4                                 # gather quadrants == AllGather chunks
# chunk boundaries in blocks: last chunk small so its AllGather (the only one
# on the critical path at a layer boundary) is short; all chunks <= 31 blocks
# so per-chunk table rows (8 cores * blocks * 128) stay under int16 range
CB0 = [0, 31, 62, 85, NBLK]
CR = [(CB0[q + 1] - CB0[q]) * BLK for q in range(NQ)]  # rows/core/chunk
CHUNK_OF_BLOCK = np.searchsorted(np.array(CB0[1:]), np.arange(NBLK), side="right")
RBLK = 4                               # dst blocks per PSUM round (4 PSUM banks)
NR = (NBLK + RBLK - 1) // RBLK         # 25 rounds
GRBLK = 4                              # dst blocks per gather call (1 round)
NGR = (NBLK + GRBLK - 1) // GRBLK      # 25 gather super-rounds
MLP_CHUNK = 512
REPS = 1

_F32 = np.float32
_BF16 = ml_dtypes.bfloat16


def _round_blocks(r):
    b0 = r * RBLK
    return range(b0, min(b0 + RBLK, NBLK))


def _gr_blocks(g):
    b0 = g * GRBLK
    return range(b0, min(b0 + GRBLK, NBLK))


def _build_structures(edge_index, batch):
    """Host-side preprocessing: per-core padded edge-slot streams.

    Returns a dict with per-core planes plus the shared capacity map.
    """
    src = np.asarray(edge_index[0], dtype=np.int64)
    dst = np.asarray(edge_index[1], dtype=np.int64)

    # src chunk assignment (by physical block; permutations below stay within
    # chunks, so this is stable under the block reordering)
    src_core = src // PER
    src_loc = src % PER
    src_blk = src_loc // BLK
    q_of_edge = CHUNK_OF_BLOCK[src_blk]

    core_of_edge = dst // PER
    dst_local = dst % PER
    b_of_edge = dst_local // BLK

    # per-core (q, physical block) counts
    counts = np.zeros((NC, NQ, NBLK), dtype=np.int64)
    np.add.at(counts, (core_of_edge, q_of_edge, b_of_edge), 1)

    # Per-core block permutation (within each chunk): sort blocks by total
    # in-degree so the rank-k block of every core has a similar count; the
    # shared capacity K = max over cores then carries far less padding.
    logpos = np.zeros((NC, NBLK), dtype=np.int64)   # physical block -> logical
    phys = np.zeros((NC, NBLK), dtype=np.int64)     # logical block -> physical
    totals = counts.sum(axis=1)                     # [NC, NBLK]
    for c in range(NC):
        for q in range(NQ):
            pbs = np.arange(CB0[q], CB0[q + 1])
            order = pbs[np.argsort(-totals[c, pbs], kind="stable")]
            logpos[c, order] = pbs
            phys[c, pbs] = order
    counts_l = np.take_along_axis(counts, phys[:, None, :], axis=2)
    K = np.ceil(counts_l.max(axis=0) / BLK).astype(np.int64)  # [NQ, NBLK] logical

    # src -> row within the per-(q, core) logical-order table stripe
    crq = np.array(CR, dtype=np.int64)
    cb0blk = np.array([CB0[q] for q in range(NQ)], dtype=np.int64)
    src_lblk = logpos[src_core, src_blk]
    qloc = (src_core * crq[q_of_edge]
            + (src_lblk - cb0blk[q_of_edge]) * BLK + src_loc % BLK).astype(np.int16)

    # dst -> logical block / relative position
    b_of_edge = logpos[core_of_edge, b_of_edge]

    # static slot-stream layout: gather super-rounds -> q -> blocks
    group_off = np.zeros((NQ, NBLK), dtype=np.int64)
    tile_of = {}      # (q, b) -> global tile index of the block's first tile
    call_meta = []    # per (g, q): (slot_offset, n_slots, queue)
    qload = [0] * NQ  # SWDGE queue load balancing (chunk sizes are uneven)
    off = 0
    for g in range(NGR):
        for q in range(NQ):
            call_off = off
            for b in _gr_blocks(g):
                group_off[q, b] = off
                tile_of[(q, b)] = off // BLK
                off += int(K[q, b]) * BLK
            # early super-rounds: pin queue==quadrant so a call waiting on the
            # last-arriving AG chunk only stalls one queue FIFO; afterwards
            # balance load (chunk sizes are uneven)
            if g < 4:
                queue = q
            else:
                queue = min(range(NQ), key=lambda i: qload[i])
            qload[queue] += off - call_off
            call_meta.append((g, q, call_off, off - call_off, queue))
    tot_slots = off
    assert tot_slots % BLK == 0
    tot_tiles = tot_slots // BLK

    idx_planes = []
    dstrel_planes = []
    ohg_planes = []
    for c in range(NC):
        sel = core_of_edge == c
        eq = q_of_edge[sel]
        eb = b_of_edge[sel]
        eloc = qloc[sel]
        edrel = (dst_local[sel] % BLK).astype(np.float32)

        idx = np.zeros(tot_slots, dtype=np.int16)
        drel = np.full(tot_slots, -1.0, dtype=np.float32)
        # order edges by (q, b); place each (q, b) run at its static offset
        order = np.lexsort((eb, eq))
        eq, eb, eloc, edrel = eq[order], eb[order], eloc[order], edrel[order]
        gid = eq * NBLK + eb
        run_starts = np.zeros(NQ * NBLK + 1, dtype=np.int64)
        np.add.at(run_starts, gid + 1, 1)
        run_starts = np.cumsum(run_starts)
        flat_off = group_off.reshape(-1)
        pos = flat_off[gid] + (np.arange(gid.size) - run_starts[gid])
        idx[pos] = eloc
        drel[pos] = edrel

        idx_plane = np.tile(idx.reshape(-1, 16).T, (8, 1))          # [128, tot/16]
        drel_plane = drel.reshape(-1, BLK).T.astype(_BF16)          # [128, tot_tiles]
        idx_planes.append(np.ascontiguousarray(idx_plane))
        dstrel_planes.append(np.ascontiguousarray(drel_plane))

        # pooling one-hots: ohg[p, b*G + g] = 1 if batch[node (c,b,p)] == g
        # (b is the LOGICAL block position after the per-core permutation)
        bfull = np.full(NBLK * BLK, -1, dtype=np.int64)
        bfull[:PER] = np.asarray(batch[c * PER:(c + 1) * PER], dtype=np.int64)
        bfull = bfull.reshape(NBLK, BLK)[phys[c]]                   # [lb, p]
        ohg = (bfull[:, :, None] == np.arange(G)[None, None, :])    # [b, p, g]
        ohg = ohg.transpose(1, 0, 2).reshape(BLK, NBLK * G).astype(_F32)
        ohg_planes.append(np.ascontiguousarray(ohg.astype(_BF16)))

    OHT = 0
    OHTQ = [0] * NQ
    for r in range(NR):
        for q in range(NQ):
            t = int(sum(K[q, b] for b in _round_blocks(r)))
            OHT = max(OHT, t)
            OHTQ[q] = max(OHTQ[q], t)

    return {
        "K": K,
        "OHT": OHT,
        "OHTQ": OHTQ,
        "tile_of": tile_of,
        "call_meta": call_meta,
        "tot_slots": tot_slots,
        "tot_tiles": tot_tiles,
        "idx_planes": idx_planes,
        "dstrel_planes": dstrel_planes,
        "ohg_planes": ohg_planes,
        "phys": phys,
    }


def _build_program(struct, eps_vals):
    import concourse.bacc as bacc
    import concourse.mybir as mybir
    from concourse import tile

    dt = mybir.dt
    AF = mybir.ActivationFunctionType
    OP = mybir.AluOpType

    tot_slots = struct["tot_slots"]
    tot_tiles = struct["tot_tiles"]
    call_meta = struct["call_meta"]

    nc = bacc.Bacc("TRN2", target_bir_lowering=False, num_swdge_queues=4)

    # ---- kernel parameters (per-core values via in_maps) ----
    xT_p = nc.declare_dram_parameter("xT", [F, PERP], dt.bfloat16, isOutput=False)
    idx_p = nc.declare_dram_parameter("idx", [128, tot_slots // 16], dt.int16, isOutput=False)
    drel_p = nc.declare_dram_parameter("dstrel", [128, tot_tiles], dt.bfloat16, isOutput=False)
    ohg_p = nc.declare_dram_parameter("ohg", [128, NBLK * G], dt.bfloat16, isOutput=False)
    OHT = struct["OHT"]
    iota2_p = nc.declare_dram_parameter("iota2", [128, OHT * 128], dt.bfloat16, isOutput=False)
    ident_p = nc.declare_dram_parameter("ident", [128, 128], dt.bfloat16, isOutput=False)
    Wp1_p = nc.declare_dram_parameter("Wp1", [F, F], dt.bfloat16, isOutput=False)
    bp1_p = nc.declare_dram_parameter("bp1", [F, 1], dt.float32, isOutput=False)
    Wp2_p = nc.declare_dram_parameter("Wp2", [F, F], dt.bfloat16, isOutput=False)
    bp2_p = nc.declare_dram_parameter("bp2", [F, 1], dt.float32, isOutput=False)
    W1_p = nc.declare_dram_parameter("W1s", [L, F, F], dt.bfloat16, isOutput=False)
    b1_p = nc.declare_dram_parameter("b1s", [L, F, 1], dt.float32, isOutput=False)
    W2_p = nc.declare_dram_parameter("W2s", [L, F, F], dt.bfloat16, isOutput=False)
    b2_p = nc.declare_dram_parameter("b2s", [L, F, 1], dt.float32, isOutput=False)
    out_p = nc.declare_dram_parameter("out", [G, L * F], dt.float32, isOutput=True)

    # ---- internal DRAM: per-chunk AG inputs + per-(layer, chunk) tables ----
    ag_in = [nc.dram_tensor(f"ag_in{q}", [CR[q], F], dt.bfloat16) for q in range(NQ)]
    tables = [
        [nc.dram_tensor(f"table{l}_{q}", [NC * CR[q], F], dt.bfloat16,
                        addr_space="Shared") for q in range(NQ)]
        for l in range(L)
    ]

    with tile.TileContext(nc) as tc:
        with (
            tc.tile_pool(name="const", bufs=1) as cpool,
            tc.tile_pool(name="ht", bufs=1) as hpool,
            tc.tile_pool(name="gath", bufs=8) as gpool,
            tc.tile_pool(name="idxp", bufs=12) as ipool,
            tc.tile_pool(name="oh0", bufs=2) as ohpool0,
            tc.tile_pool(name="oh1", bufs=2) as ohpool1,
            tc.tile_pool(name="oh2", bufs=2) as ohpool2,
            tc.tile_pool(name="oh3", bufs=2) as ohpool3,
            tc.tile_pool(name="zz", bufs=2) as zpool,
            tc.tile_pool(name="emit", bufs=4) as epool,
            tc.tile_pool(name="psag", bufs=5, space="PSUM") as psag,
            tc.tile_pool(name="psmlp", bufs=1, space="PSUM") as psmlp,
            tc.tile_pool(name="pstr", bufs=1, space="PSUM") as pstr,
            tc.tile_pool(name="pspool", bufs=1, space="PSUM") as pspool,
        ):
            # ---- load constants / weights ----
            iota2_sb = cpool.tile([128, OHT, 128], dt.bfloat16, tag="iota2")
            nc.sync.dma_start(iota2_sb[:].rearrange("p a b -> p (a b)"), iota2_p[:])
            ident_sb = cpool.tile([128, 128], dt.bfloat16, tag="ident")
            nc.sync.dma_start(ident_sb[:], ident_p[:])
            ohg_sb = cpool.tile([128, NBLK * G], dt.bfloat16, tag="ohg")
            nc.sync.dma_start(ohg_sb[:], ohg_p[:])
            drel_sb = cpool.tile([128, tot_tiles], dt.bfloat16, tag="drel")
            nc.sync.dma_start(drel_sb[:], drel_p[:])

            def _load_w(tag, pslice):
                t = cpool.tile([F, F], dt.bfloat16, tag=tag)
                nc.sync.dma_start(t[:], pslice)
                return t

            def _load_b(tag, pslice):
                t = cpool.tile([F, 1], dt.float32, tag=tag)
                nc.sync.dma_start(t[:], pslice)
                return t

            Wp1 = _load_w("Wp1", Wp1_p[:])
            Wp2 = _load_w("Wp2", Wp2_p[:])
            bp1 = _load_b("bp1", bp1_p[:])
            bp2 = _load_b("bp2", bp2_p[:])
            W1 = [_load_w(f"W1_{l}", W1_p[l][:]) for l in range(L)]
            W2 = [_load_w(f"W2_{l}", W2_p[l][:]) for l in range(L)]
            b1 = [_load_b(f"b1_{l}", b1_p[l][:]) for l in range(L)]
            b2 = [_load_b(f"b2_{l}", b2_p[l][:]) for l in range(L)]

            hT = hpool.tile([F, PERP], dt.bfloat16, tag="hT")

            ohpools = [ohpool0, ohpool1, ohpool2, ohpool3]
            KC = struct["K"]
            OHTQ = struct["OHTQ"]
            tile_of = struct["tile_of"]

            def _do_round(l, r, gts, vis_done, mlp_chunk):
                """One PSUM round: one-hots, segment-sum matmuls, MLP+emit."""
                rblocks = [b for b in _round_blocks(r) if b < NBLK]
                if not rblocks:
                    return
                ohs = {}
                for q in range(NQ):
                    t0 = tile_of.get((q, rblocks[0]))
                    Tr = sum(int(KC[q, b]) for b in rblocks)
                    if Tr == 0 or q not in gts:
                        continue
                    oh = ohpools[q].tile([128, OHTQ[q], 128], dt.bfloat16,
                                         tag="oh", name=f"oh_l{l}_r{r}_q{q}")
                    nc.vector.tensor_tensor(
                        oh[:, :Tr, :],
                        drel_sb[:, t0:t0 + Tr].unsqueeze(2)
                            .broadcast_to([128, Tr, 128]),
                        iota2_sb[:, :Tr, :],
                        OP.is_equal,
                    )
                    ohs[q] = (oh, t0)
                # per-block PSUM accumulators, one full bank each
                agg_of = {}
                for b in rblocks:
                    agg_of[b] = psag.tile([F, BLK], dt.float32, tag="agg",
                                          name=f"agg_l{l}_b{b}")
                    if int(KC[:, b].sum()) == 0:
                        nc.vector.memset(agg_of[b][:], 0.0)
                for q in range(NQ):
                    if q not in ohs:
                        continue
                    oh, t0 = ohs[q]
                    gt, c0 = gts[q]
                    for b in rblocks:
                        nvis = int(KC[:, b].sum())
                        bt = tile_of[(q, b)]
                        for t in range(int(KC[q, b])):
                            nc.tensor.matmul(
                                agg_of[b][:],
                                gt[:, bt - c0 + t, :],
                                oh[:, bt - t0 + t, :],
                                start=(vis_done[b] == 0),
                                stop=(vis_done[b] == nvis - 1),
                                skip_group_check=True,
                            )
                            vis_done[b] += 1
                # close the round: z, MLP, emit (one chunk per round)
                o = rblocks[0] * BLK
                mlp_chunk(o, (rblocks[-1] + 1) * BLK - o, agg_of)

            def _issue_ag(l_out, q):
                """AllGather chunk q of layer l_out's table."""
                nc.gpsimd.collective_compute(
                    "AllGather", OP.bypass,
                    replica_groups=[list(range(NC))],
                    ins=[ag_in[q][:]], outs=[tables[l_out][q][:]],
                )

            for _rep in range(REPS):
                pool_psums = []

                def _emit_block(b, layer_out):
                    """Transpose block b of hT; DMA to its ag_in chunk (if a
                    table is still needed) and accumulate pooling (l_out>=1)."""
                    ptr = pstr.tile([128, 128], dt.bfloat16, tag="tr")
                    nc.tensor.transpose(ptr[:], hT[:, b * BLK:(b + 1) * BLK], ident_sb[:])
                    hrow = epool.tile([128, 128], dt.bfloat16, tag="hrow")
                    nc.scalar.activation(hrow[:], ptr[:], AF.Copy)
                    if layer_out < L:
                        q = int(CHUNK_OF_BLOCK[b])
                        rb = (b - CB0[q]) * BLK
                        nc.sync.dma_start(ag_in[q][rb:rb + BLK, :], hrow[:])
                    if layer_out >= 1:
                        nc.tensor.matmul(
                            pool_psums[layer_out - 1][:],
                            ohg_sb[:, b * G:(b + 1) * G],
                            hrow[:],
                            start=(b == 0),
                            stop=(b == NBLK - 1),
                            skip_group_check=True,
                        )

                # ---- pre-MLP: hT = relu(relu(x Wp1 + bp1) Wp2 + bp2), transposed,
                # fused with per-block emit into table0 chunks (AG each chunk
                # as soon as its last block is emitted)
                o = 0
                while o < PERP:
                    cw = min(MLP_CHUNK, PERP - o)
                    xc = zpool.tile([F, MLP_CHUNK], dt.bfloat16, tag="xc")
                    nc.sync.dma_start(xc[:, :cw], xT_p[:, o:o + cw])
                    p1 = psmlp.tile([F, MLP_CHUNK], dt.float32, tag="mlp")
                    nc.tensor.matmul(p1[:, :cw], Wp1[:], xc[:, :cw])
                    t1 = zpool.tile([F, MLP_CHUNK], dt.bfloat16, tag="t1")
                    nc.scalar.activation(t1[:, :cw], p1[:, :cw], AF.Relu, bias=bp1[:])
                    p2 = psmlp.tile([F, MLP_CHUNK], dt.float32, tag="mlp")
                    nc.tensor.matmul(p2[:, :cw], Wp2[:], t1[:, :cw])
                    nc.scalar.activation(hT[:, o:o + cw], p2[:, :cw], AF.Relu, bias=bp2[:])
                    for b in range(o // BLK, (o + cw) // BLK):
                        _emit_block(b, 0)
                        if b + 1 in CB0:
                            _issue_ag(0, int(CHUNK_OF_BLOCK[b]))
                    o += cw

                # ---- GIN layers ----
                for l in range(L):
                    pool_psums.append(pspool.tile([G, F], dt.float32, tag="pool", name=f"poolp{l}"))
                    # prescale: hT *= (1 + eps_l)   (table_l already captured h_l)
                    nc.vector.tensor_scalar(
                        hT[:], hT[:], float(1.0 + eps_vals[l]), None, op0=OP.mult
                    )

                    K = struct["K"]

                    def _mlp_chunk(o, cw, agg_of):
                        z = zpool.tile([F, MLP_CHUNK], dt.bfloat16, tag="z",
                                       name=f"z_l{l}_o{o}")
                        for k in range(cw // BLK):
                            b = o // BLK + k
                            nc.vector.tensor_tensor(
                                z[:, k * BLK:(k + 1) * BLK],
                                agg_of[b][:],
                                hT[:, b * BLK:(b + 1) * BLK],
                                OP.add,
                            )
                        p1 = psmlp.tile([F, MLP_CHUNK], dt.float32, tag="mlp",
                                        name=f"p1_l{l}_o{o}")
                        nc.tensor.matmul(p1[:, :cw], W1[l][:], z[:, :cw])
                        t1 = zpool.tile([F, MLP_CHUNK], dt.bfloat16, tag="t1",
                                        name=f"t1_l{l}_o{o}")
                        nc.scalar.activation(t1[:, :cw], p1[:, :cw], AF.Relu, bias=b1[l][:])
                        p2 = psmlp.tile([F, MLP_CHUNK], dt.float32, tag="mlp",
                                        name=f"p2_l{l}_o{o}")
                        nc.tensor.matmul(p2[:, :cw], W2[l][:], t1[:, :cw])
                        nc.scalar.activation(hT[:, o:o + cw], p2[:, :cw], AF.Identity,
                                             bias=b2[l][:])
                        for k in range(cw // BLK):
                            b = o // BLK + k
                            _emit_block(b, l + 1)
                            if l + 1 < L and b + 1 in CB0:
                                _issue_ag(l + 1, int(CHUNK_OF_BLOCK[b]))

                    tile_of = struct["tile_of"]
                    vis_done = {b: 0 for b in range(NBLK)}
                    for g in range(NGR):
                        # issue big gathers (one per quadrant, spanning GRBLK
                        # blocks), each on its own SWDGE queue so all four Q7
                        # core-pairs generate descriptors concurrently
                        gts = {}
                        for (gg, q, call_off, n_slots, queue) in call_meta:
                            if gg != g or n_slots == 0:
                                continue
                            T = n_slots // BLK
                            idxs = ipool.tile([128, n_slots // 16], dt.int16,
                                              tag="idxs", name=f"idxs_l{l}_g{g}_q{q}")
                            nc.sync.dma_start(
                                idxs[:], idx_p[:, call_off // 16:(call_off + n_slots) // 16]
                            )
                            gt = gpool.tile([128, T, 128], dt.bfloat16, tag="gt",
                                            name=f"gt_l{l}_g{g}_q{q}")
                            nc.gpsimd.dma_gather(
                                gt[:],
                                tables[l][q][:],
                                idxs[:],
                                n_slots,
                                n_slots,
                                F,
                                single_packet=False,
                                queue_num=queue,
                            )
                            gts[q] = (gt, call_off // BLK)

                        for r in range(g * GRBLK // RBLK,
                                       min((g + 1) * GRBLK, NBLK + RBLK - 1) // RBLK):
                            _do_round(l, r, gts, vis_done, _mlp_chunk)

                    # extract pooled sums for this layer
                    pooled_sb = epool.tile([G, F], dt.float32, tag="pooled")
                    nc.scalar.activation(pooled_sb[:], pool_psums[l][:], AF.Copy)
                    nc.sync.dma_start(out_p[:, l * F:(l + 1) * F], pooled_sb[:])

    nc.compile()
    return nc


def _make_in_maps(struct, inputs):
    x = np.asarray(inputs["x"], dtype=_F32)
    OHT = struct["OHT"]
    # iota2[p, t*128 + j] = j  (contiguous one-hot layout [128, OHT, 128])
    iota2 = np.broadcast_to(
        np.arange(128, dtype=_F32)[None, :], (OHT, 128)
    ).reshape(OHT * 128)
    iota2 = np.broadcast_to(iota2[None, :], (128, OHT * 128)).astype(_BF16)
    ident = np.eye(128, dtype=_F32).astype(_BF16)

    shared = {
        "iota2": np.ascontiguousarray(iota2),
        "ident": np.ascontiguousarray(ident),
        "Wp1": np.asarray(inputs["W_pre1"], dtype=_F32).astype(_BF16),
        "bp1": np.asarray(inputs["b_pre1"], dtype=_F32).reshape(F, 1),
        "Wp2": np.asarray(inputs["W_pre2"], dtype=_F32).astype(_BF16),
        "bp2": np.asarray(inputs["b_pre2"], dtype=_F32).reshape(F, 1),
        "W1s": np.asarray(inputs["W1s"], dtype=_F32).astype(_BF16),
        "b1s": np.asarray(inputs["b1s"], dtype=_F32).reshape(L, F, 1),
        "W2s": np.asarray(inputs["W2s"], dtype=_F32).astype(_BF16),
        "b2s": np.asarray(inputs["b2s"], dtype=_F32).reshape(L, F, 1),
    }

    phys = struct["phys"]
    in_maps = []
    for c in range(NC):
        xs = np.zeros((F, PERP), dtype=_F32)
        xs[:, :PER] = x[c * PER:(c + 1) * PER].T
        xs = xs.reshape(F, NBLK, BLK)[:, phys[c], :].reshape(F, PERP)
        m = dict(shared)
        m["xT"] = np.ascontiguousarray(xs.astype(_BF16))
        m["idx"] = struct["idx_planes"][c]
        m["dstrel"] = struct["dstrel_planes"][c]
        m["ohg"] = struct["ohg_planes"][c]
        in_maps.append(m)
    return in_maps


def kernel(**inputs):
    from concourse.bass_utils import run_bass_kernel_spmd

    edge_index = np.asarray(inputs["edge_index"])
    batch = np.asarray(inputs["batch"])
    eps = np.asarray(inputs["eps"], dtype=_F32)

    struct = _build_structures(edge_index, batch)
    nc = _build_program(struct, [float(e) for e in eps])
    in_maps = _make_in_maps(struct, inputs)

    res = run_bass_kernel_spmd(nc, in_maps, core_ids=list(range(NC)))
    out = np.zeros((G, L * F), dtype=_F32)
    for c in range(NC):
        out += res.results[c]["out"]
    return out


# revision 27
# speedup vs baseline: 1.0390x; 1.0390x over previous
"""Distributed GIN (3-layer) kernel for 8 TRN2 NeuronCores.

Sharding: nodes (and their incident in-edges) are partitioned across the 8
cores; each core keeps its node-feature shard resident in SBUF (transposed,
bf16).  Between layers the per-core shards are AllGathered into bf16
"gather tables" in DRAM — chunked 4 ways, with gather QUADRANT == AG CHUNK,
so each chunk's AllGather is issued as soon as its blocks are computed and
overlaps the remaining compute; the next layer's gathers per quadrant wait
only on their own chunk.  Neighbor aggregation is a dma_gather (256B/edge,
spread across 4 SWDGE queues so all four Q7 core-pairs emit descriptors in
parallel) + one-hot matmul segment-sum on the tensor engine.  MLP weights
are replicated (bf16).  Per-graph pooled sums are computed per-core on the
PE (host-precomputed one-hot pooling masks) and reduced on the host.
"""

import math

import ml_dtypes
import numpy as np

N_NODES = 100000
N_EDGES = 1600000
F = 128              # feature dim (= hidden dim)
L = 3                # GIN layers
G = 64               # graphs
NC = 8               # NeuronCores
PER = N_NODES // NC  # 12500 real nodes per core
BLK = 128
NBLK = (PER + BLK - 1) // BLK          # 98 node blocks per core
PERP = NBLK * BLK                      # 12544 padded rows per core
NQ = 4                                 # gather quadrants == AllGather chunks
# chunk boundaries in blocks: last chunk small so its AllGather (the only one
# on the critical path at a layer boundary) is short; all chunks <= 31 blocks
# so per-chunk table rows (8 cores * blocks * 128) stay under int16 range
CB0 = [0, 31, 62, 85, NBLK]
CR = [(CB0[q + 1] - CB0[q]) * BLK for q in range(NQ)]  # rows/core/chunk
CHUNK_OF_BLOCK = np.searchsorted(np.array(CB0[1:]), np.arange(NBLK), side="right")
RBLK = 4                               # dst blocks per PSUM round (4 PSUM banks)
NR = (NBLK + RBLK - 1) // RBLK         # 25 rounds
GRBLK = 4                              # dst blocks per gather call (1 round)
NGR = (NBLK + GRBLK - 1) // GRBLK      # 25 gather super-rounds
MLP_CHUNK = 512
REPS = 1

_F32 = np.float32
_BF16 = ml_dtypes.bfloat16


def _round_blocks(r):
    b0 = r * RBLK
    return range(b0, min(b0 + RBLK, NBLK))


def _gr_blocks(g):
    b0 = g * GRBLK
    return range(b0, min(b0 + GRBLK, NBLK))


def _build_structures(edge_index, batch):
    """Host-side preprocessing: per-core padded edge-slot streams.

    Returns a dict with per-core planes plus the shared capacity map.
    """
    src = np.asarray(edge_index[0], dtype=np.int64)
    dst = np.asarray(edge_index[1], dtype=np.int64)

    # src chunk assignment (by physical block; permutations below stay within
    # chunks, so this is stable under the block reordering)
    src_core = src // PER
    src_loc = src % PER
    src_blk = src_loc // BLK
    q_of_edge = CHUNK_OF_BLOCK[src_blk]

    core_of_edge = dst // PER
    dst_local = dst % PER
    b_of_edge = dst_local // BLK

    # per-core (q, physical block) counts
    counts = np.zeros((NC, NQ, NBLK), dtype=np.int64)
    np.add.at(counts, (core_of_edge, q_of_edge, b_of_edge), 1)

    # Per-core block permutation (within each chunk): sort blocks by total
    # in-degree so the rank-k block of every core has a similar count; the
    # shared capacity K = max over cores then carries far less padding.
    logpos = np.zeros((NC, NBLK), dtype=np.int64)   # physical block -> logical
    phys = np.zeros((NC, NBLK), dtype=np.int64)     # logical block -> physical
    totals = counts.sum(axis=1)                     # [NC, NBLK]
    for c in range(NC):
        for q in range(NQ):
            pbs = np.arange(CB0[q], CB0[q + 1])
            order = pbs[np.argsort(-totals[c, pbs], kind="stable")]
            logpos[c, order] = pbs
            phys[c, pbs] = order
    counts_l = np.take_along_axis(counts, phys[:, None, :], axis=2)
    K = np.ceil(counts_l.max(axis=0) / BLK).astype(np.int64)  # [NQ, NBLK] logical

    # src -> row within the per-(q, core) logical-order table stripe
    crq = np.array(CR, dtype=np.int64)
    cb0blk = np.array([CB0[q] for q in range(NQ)], dtype=np.int64)
    src_lblk = logpos[src_core, src_blk]
    qloc = (src_core * crq[q_of_edge]
            + (src_lblk - cb0blk[q_of_edge]) * BLK + src_loc % BLK).astype(np.int16)

    # dst -> logical block / relative position
    b_of_edge = logpos[core_of_edge, b_of_edge]

    # static slot-stream layout: gather super-rounds -> q -> blocks
    group_off = np.zeros((NQ, NBLK), dtype=np.int64)
    tile_of = {}      # (q, b) -> global tile index of the block's first tile
    call_meta = []    # per (g, q): (slot_offset, n_slots, queue)
    qload = [0] * NQ  # SWDGE queue load balancing (chunk sizes are uneven)
    off = 0
    for g in range(NGR):
        for q in range(NQ):
            call_off = off
            for b in _gr_blocks(g):
                group_off[q, b] = off
                tile_of[(q, b)] = off // BLK
                off += int(K[q, b]) * BLK
            queue = min(range(NQ), key=lambda i: qload[i])
            qload[queue] += off - call_off
            call_meta.append((g, q, call_off, off - call_off, queue))
    tot_slots = off
    assert tot_slots % BLK == 0
    tot_tiles = tot_slots // BLK

    idx_planes = []
    dstrel_planes = []
    ohg_planes = []
    for c in range(NC):
        sel = core_of_edge == c
        eq = q_of_edge[sel]
        eb = b_of_edge[sel]
        eloc = qloc[sel]
        edrel = (dst_local[sel] % BLK).astype(np.float32)

        idx = np.zeros(tot_slots, dtype=np.int16)
        drel = np.full(tot_slots, -1.0, dtype=np.float32)
        # order edges by (q, b); place each (q, b) run at its static offset
        order = np.lexsort((eb, eq))
        eq, eb, eloc, edrel = eq[order], eb[order], eloc[order], edrel[order]
        gid = eq * NBLK + eb
        run_starts = np.zeros(NQ * NBLK + 1, dtype=np.int64)
        np.add.at(run_starts, gid + 1, 1)
        run_starts = np.cumsum(run_starts)
        flat_off = group_off.reshape(-1)
        pos = flat_off[gid] + (np.arange(gid.size) - run_starts[gid])
        idx[pos] = eloc
        drel[pos] = edrel

        idx_plane = np.tile(idx.reshape(-1, 16).T, (8, 1))          # [128, tot/16]
        drel_plane = drel.reshape(-1, BLK).T.astype(_BF16)          # [128, tot_tiles]
        idx_planes.append(np.ascontiguousarray(idx_plane))
        dstrel_planes.append(np.ascontiguousarray(drel_plane))

        # pooling one-hots: ohg[p, b*G + g] = 1 if batch[node (c,b,p)] == g
        # (b is the LOGICAL block position after the per-core permutation)
        bfull = np.full(NBLK * BLK, -1, dtype=np.int64)
        bfull[:PER] = np.asarray(batch[c * PER:(c + 1) * PER], dtype=np.int64)
        bfull = bfull.reshape(NBLK, BLK)[phys[c]]                   # [lb, p]
        ohg = (bfull[:, :, None] == np.arange(G)[None, None, :])    # [b, p, g]
        ohg = ohg.transpose(1, 0, 2).reshape(BLK, NBLK * G).astype(_F32)
        ohg_planes.append(np.ascontiguousarray(ohg.astype(_BF16)))

    OHT = 0
    OHTQ = [0] * NQ
    for r in range(NR):
        for q in range(NQ):
            t = int(sum(K[q, b] for b in _round_blocks(r)))
            OHT = max(OHT, t)
            OHTQ[q] = max(OHTQ[q], t)

    return {
        "K": K,
        "OHT": OHT,
        "OHTQ": OHTQ,
        "tile_of": tile_of,
        "call_meta": call_meta,
        "tot_slots": tot_slots,
        "tot_tiles": tot_tiles,
        "idx_planes": idx_planes,
        "dstrel_planes": dstrel_planes,
        "ohg_planes": ohg_planes,
        "phys": phys,
    }


def _build_program(struct, eps_vals):
    import concourse.bacc as bacc
    import concourse.mybir as mybir
    from concourse import tile

    dt = mybir.dt
    AF = mybir.ActivationFunctionType
    OP = mybir.AluOpType

    tot_slots = struct["tot_slots"]
    tot_tiles = struct["tot_tiles"]
    call_meta = struct["call_meta"]

    nc = bacc.Bacc("TRN2", target_bir_lowering=False, num_swdge_queues=4)

    # ---- kernel parameters (per-core values via in_maps) ----
    xT_p = nc.declare_dram_parameter("xT", [F, PERP], dt.bfloat16, isOutput=False)
    idx_p = nc.declare_dram_parameter("idx", [128, tot_slots // 16], dt.int16, isOutput=False)
    drel_p = nc.declare_dram_parameter("dstrel", [128, tot_tiles], dt.bfloat16, isOutput=False)
    ohg_p = nc.declare_dram_parameter("ohg", [128, NBLK * G], dt.bfloat16, isOutput=False)
    OHT = struct["OHT"]
    iota2_p = nc.declare_dram_parameter("iota2", [128, OHT * 128], dt.bfloat16, isOutput=False)
    ident_p = nc.declare_dram_parameter("ident", [128, 128], dt.bfloat16, isOutput=False)
    Wp1_p = nc.declare_dram_parameter("Wp1", [F, F], dt.bfloat16, isOutput=False)
    bp1_p = nc.declare_dram_parameter("bp1", [F, 1], dt.float32, isOutput=False)
    Wp2_p = nc.declare_dram_parameter("Wp2", [F, F], dt.bfloat16, isOutput=False)
    bp2_p = nc.declare_dram_parameter("bp2", [F, 1], dt.float32, isOutput=False)
    W1_p = nc.declare_dram_parameter("W1s", [L, F, F], dt.bfloat16, isOutput=False)
    b1_p = nc.declare_dram_parameter("b1s", [L, F, 1], dt.float32, isOutput=False)
    W2_p = nc.declare_dram_parameter("W2s", [L, F, F], dt.bfloat16, isOutput=False)
    b2_p = nc.declare_dram_parameter("b2s", [L, F, 1], dt.float32, isOutput=False)
    out_p = nc.declare_dram_parameter("out", [G, L * F], dt.float32, isOutput=True)

    # ---- internal DRAM: per-chunk AG inputs + per-(layer, chunk) tables ----
    ag_in = [nc.dram_tensor(f"ag_in{q}", [CR[q], F], dt.bfloat16) for q in range(NQ)]
    tables = [
        [nc.dram_tensor(f"table{l}_{q}", [NC * CR[q], F], dt.bfloat16,
                        addr_space="Shared") for q in range(NQ)]
        for l in range(L)
    ]

    with tile.TileContext(nc) as tc:
        with (
            tc.tile_pool(name="const", bufs=1) as cpool,
            tc.tile_pool(name="ht", bufs=1) as hpool,
            tc.tile_pool(name="gath", bufs=10) as gpool,
            tc.tile_pool(name="idxp", bufs=12) as ipool,
            tc.tile_pool(name="oh0", bufs=2) as ohpool0,
            tc.tile_pool(name="oh1", bufs=2) as ohpool1,
            tc.tile_pool(name="oh2", bufs=2) as ohpool2,
            tc.tile_pool(name="oh3", bufs=2) as ohpool3,
            tc.tile_pool(name="zz", bufs=2) as zpool,
            tc.tile_pool(name="emit", bufs=4) as epool,
            tc.tile_pool(name="psag", bufs=4, space="PSUM") as psag,
            tc.tile_pool(name="psmlp", bufs=2, space="PSUM") as psmlp,
            tc.tile_pool(name="pstr", bufs=1, space="PSUM") as pstr,
            tc.tile_pool(name="pspool", bufs=1, space="PSUM") as pspool,
        ):
            # ---- load constants / weights ----
            iota2_sb = cpool.tile([128, OHT, 128], dt.bfloat16, tag="iota2")
            nc.sync.dma_start(iota2_sb[:].rearrange("p a b -> p (a b)"), iota2_p[:])
            ident_sb = cpool.tile([128, 128], dt.bfloat16, tag="ident")
            nc.sync.dma_start(ident_sb[:], ident_p[:])
            ohg_sb = cpool.tile([128, NBLK * G], dt.bfloat16, tag="ohg")
            nc.sync.dma_start(ohg_sb[:], ohg_p[:])
            drel_sb = cpool.tile([128, tot_tiles], dt.bfloat16, tag="drel")
            nc.sync.dma_start(drel_sb[:], drel_p[:])

            def _load_w(tag, pslice):
                t = cpool.tile([F, F], dt.bfloat16, tag=tag)
                nc.sync.dma_start(t[:], pslice)
                return t

            def _load_b(tag, pslice):
                t = cpool.tile([F, 1], dt.float32, tag=tag)
                nc.sync.dma_start(t[:], pslice)
                return t

            Wp1 = _load_w("Wp1", Wp1_p[:])
            Wp2 = _load_w("Wp2", Wp2_p[:])
            bp1 = _load_b("bp1", bp1_p[:])
            bp2 = _load_b("bp2", bp2_p[:])
            W1 = [_load_w(f"W1_{l}", W1_p[l][:]) for l in range(L)]
            W2 = [_load_w(f"W2_{l}", W2_p[l][:]) for l in range(L)]
            b1 = [_load_b(f"b1_{l}", b1_p[l][:]) for l in range(L)]
            b2 = [_load_b(f"b2_{l}", b2_p[l][:]) for l in range(L)]

            hT = hpool.tile([F, PERP], dt.bfloat16, tag="hT")

            ohpools = [ohpool0, ohpool1, ohpool2, ohpool3]
            KC = struct["K"]
            OHTQ = struct["OHTQ"]
            tile_of = struct["tile_of"]

            def _do_round(l, r, gts, vis_done, mlp_chunk):
                """One PSUM round: one-hots, segment-sum matmuls, MLP+emit."""
                rblocks = [b for b in _round_blocks(r) if b < NBLK]
                if not rblocks:
                    return
                ohs = {}
                for q in range(NQ):
                    t0 = tile_of.get((q, rblocks[0]))
                    Tr = sum(int(KC[q, b]) for b in rblocks)
                    if Tr == 0 or q not in gts:
                        continue
                    oh = ohpools[q].tile([128, OHTQ[q], 128], dt.bfloat16,
                                         tag="oh", name=f"oh_l{l}_r{r}_q{q}")
                    nc.vector.tensor_tensor(
                        oh[:, :Tr, :],
                        drel_sb[:, t0:t0 + Tr].unsqueeze(2)
                            .broadcast_to([128, Tr, 128]),
                        iota2_sb[:, :Tr, :],
                        OP.is_equal,
                    )
                    ohs[q] = (oh, t0)
                # per-block PSUM accumulators, one full bank each
                agg_of = {}
                for b in rblocks:
                    agg_of[b] = psag.tile([F, BLK], dt.float32, tag="agg",
                                          name=f"agg_l{l}_b{b}")
                    if int(KC[:, b].sum()) == 0:
                        nc.vector.memset(agg_of[b][:], 0.0)
                for q in range(NQ):
                    if q not in ohs:
                        continue
                    oh, t0 = ohs[q]
                    gt, c0 = gts[q]
                    for b in rblocks:
                        nvis = int(KC[:, b].sum())
                        bt = tile_of[(q, b)]
                        for t in range(int(KC[q, b])):
                            nc.tensor.matmul(
                                agg_of[b][:],
                                gt[:, bt - c0 + t, :],
                                oh[:, bt - t0 + t, :],
                                start=(vis_done[b] == 0),
                                stop=(vis_done[b] == nvis - 1),
                                skip_group_check=True,
                            )
                            vis_done[b] += 1
                # close the round: z, MLP, emit (one chunk per round)
                o = rblocks[0] * BLK
                mlp_chunk(o, (rblocks[-1] + 1) * BLK - o, agg_of)

            def _issue_ag(l_out, q):
                """AllGather chunk q of layer l_out's table."""
                nc.gpsimd.collective_compute(
                    "AllGather", OP.bypass,
                    replica_groups=[list(range(NC))],
                    ins=[ag_in[q][:]], outs=[tables[l_out][q][:]],
                )

            for _rep in range(REPS):
                pool_psums = []

                def _emit_block(b, layer_out):
                    """Transpose block b of hT; DMA to its ag_in chunk (if a
                    table is still needed) and accumulate pooling (l_out>=1)."""
                    ptr = pstr.tile([128, 128], dt.bfloat16, tag="tr")
                    nc.tensor.transpose(ptr[:], hT[:, b * BLK:(b + 1) * BLK], ident_sb[:])
                    hrow = epool.tile([128, 128], dt.bfloat16, tag="hrow")
                    nc.scalar.activation(hrow[:], ptr[:], AF.Copy)
                    if layer_out < L:
                        q = int(CHUNK_OF_BLOCK[b])
                        rb = (b - CB0[q]) * BLK
                        nc.sync.dma_start(ag_in[q][rb:rb + BLK, :], hrow[:])
                    if layer_out >= 1:
                        nc.tensor.matmul(
                            pool_psums[layer_out - 1][:],
                            ohg_sb[:, b * G:(b + 1) * G],
                            hrow[:],
                            start=(b == 0),
                            stop=(b == NBLK - 1),
                            skip_group_check=True,
                        )

                # ---- pre-MLP: hT = relu(relu(x Wp1 + bp1) Wp2 + bp2), transposed,
                # fused with per-block emit into table0 chunks (AG each chunk
                # as soon as its last block is emitted)
                o = 0
                while o < PERP:
                    cw = min(MLP_CHUNK, PERP - o)
                    xc = zpool.tile([F, MLP_CHUNK], dt.bfloat16, tag="xc")
                    nc.sync.dma_start(xc[:, :cw], xT_p[:, o:o + cw])
                    p1 = psmlp.tile([F, MLP_CHUNK], dt.float32, tag="mlp")
                    nc.tensor.matmul(p1[:, :cw], Wp1[:], xc[:, :cw])
                    t1 = zpool.tile([F, MLP_CHUNK], dt.bfloat16, tag="t1")
                    nc.scalar.activation(t1[:, :cw], p1[:, :cw], AF.Relu, bias=bp1[:])
                    p2 = psmlp.tile([F, MLP_CHUNK], dt.float32, tag="mlp")
                    nc.tensor.matmul(p2[:, :cw], Wp2[:], t1[:, :cw])
                    nc.scalar.activation(hT[:, o:o + cw], p2[:, :cw], AF.Relu, bias=bp2[:])
                    for b in range(o // BLK, (o + cw) // BLK):
                        _emit_block(b, 0)
                        if b + 1 in CB0:
                            _issue_ag(0, int(CHUNK_OF_BLOCK[b]))
                    o += cw

                # ---- GIN layers ----
                for l in range(L):
                    pool_psums.append(pspool.tile([G, F], dt.float32, tag="pool", name=f"poolp{l}"))
                    # prescale: hT *= (1 + eps_l)   (table_l already captured h_l)
                    nc.vector.tensor_scalar(
                        hT[:], hT[:], float(1.0 + eps_vals[l]), None, op0=OP.mult
                    )

                    K = struct["K"]

                    def _mlp_chunk(o, cw, agg_of):
                        z = zpool.tile([F, MLP_CHUNK], dt.bfloat16, tag="z",
                                       name=f"z_l{l}_o{o}")
                        for k in range(cw // BLK):
                            b = o // BLK + k
                            nc.vector.tensor_tensor(
                                z[:, k * BLK:(k + 1) * BLK],
                                agg_of[b][:],
                                hT[:, b * BLK:(b + 1) * BLK],
                                OP.add,
                            )
                        p1 = psmlp.tile([F, MLP_CHUNK], dt.float32, tag="mlp",
                                        name=f"p1_l{l}_o{o}")
                        nc.tensor.matmul(p1[:, :cw], W1[l][:], z[:, :cw])
                        t1 = zpool.tile([F, MLP_CHUNK], dt.bfloat16, tag="t1",
                                        name=f"t1_l{l}_o{o}")
                        nc.scalar.activation(t1[:, :cw], p1[:, :cw], AF.Relu, bias=b1[l][:])
                        p2 = psmlp.tile([F, MLP_CHUNK], dt.float32, tag="mlp",
                                        name=f"p2_l{l}_o{o}")
                        nc.tensor.matmul(p2[:, :cw], W2[l][:], t1[:, :cw])
                        nc.scalar.activation(hT[:, o:o + cw], p2[:, :cw], AF.Identity,
                                             bias=b2[l][:])
                        for k in range(cw // BLK):
                            b = o // BLK + k
                            _emit_block(b, l + 1)
                            if l + 1 < L and b + 1 in CB0:
                                _issue_ag(l + 1, int(CHUNK_OF_BLOCK[b]))

                    tile_of = struct["tile_of"]
                    vis_done = {b: 0 for b in range(NBLK)}
                    for g in range(NGR):
                        # issue big gathers (one per quadrant, spanning GRBLK
                        # blocks), each on its own SWDGE queue so all four Q7
                        # core-pairs generate descriptors concurrently
                        gts = {}
                        for (gg, q, call_off, n_slots, queue) in call_meta:
                            if gg != g or n_slots == 0:
                                continue
                            T = n_slots // BLK
                            idxs = ipool.tile([128, n_slots // 16], dt.int16,
                                              tag="idxs", name=f"idxs_l{l}_g{g}_q{q}")
                            nc.sync.dma_start(
                                idxs[:], idx_p[:, call_off // 16:(call_off + n_slots) // 16]
                            )
                            gt = gpool.tile([128, T, 128], dt.bfloat16, tag="gt",
                                            name=f"gt_l{l}_g{g}_q{q}")
                            nc.gpsimd.dma_gather(
                                gt[:],
                                tables[l][q][:],
                                idxs[:],
                                n_slots,
                                n_slots,
                                F,
                                single_packet=False,
                                queue_num=queue,
                            )
                            gts[q] = (gt, call_off // BLK)

                        for r in range(g * GRBLK // RBLK,
                                       min((g + 1) * GRBLK, NBLK + RBLK - 1) // RBLK):
                            _do_round(l, r, gts, vis_done, _mlp_chunk)

                    # extract pooled sums for this layer
                    pooled_sb = epool.tile([G, F], dt.float32, tag="pooled")
                    nc.scalar.activation(pooled_sb[:], pool_psums[l][:], AF.Copy)
                    nc.sync.dma_start(out_p[:, l * F:(l + 1) * F], pooled_sb[:])

    nc.compile()
    return nc


def _make_in_maps(struct, inputs):
    x = np.asarray(inputs["x"], dtype=_F32)
    OHT = struct["OHT"]
    # iota2[p, t*128 + j] = j  (contiguous one-hot layout [128, OHT, 128])
    iota2 = np.broadcast_to(
        np.arange(128, dtype=_F32)[None, :], (OHT, 128)
    ).reshape(OHT * 128)
    iota2 = np.broadcast_to(iota2[None, :], (128, OHT * 128)).astype(_BF16)
    ident = np.eye(128, dtype=_F32).astype(_BF16)

    shared = {
        "iota2": np.ascontiguousarray(iota2),
        "ident": np.ascontiguousarray(ident),
        "Wp1": np.asarray(inputs["W_pre1"], dtype=_F32).astype(_BF16),
        "bp1": np.asarray(inputs["b_pre1"], dtype=_F32).reshape(F, 1),
        "Wp2": np.asarray(inputs["W_pre2"], dtype=_F32).astype(_BF16),
        "bp2": np.asarray(inputs["b_pre2"], dtype=_F32).reshape(F, 1),
        "W1s": np.asarray(inputs["W1s"], dtype=_F32).astype(_BF16),
        "b1s": np.asarray(inputs["b1s"], dtype=_F32).reshape(L, F, 1),
        "W2s": np.asarray(inputs["W2s"], dtype=_F32).astype(_BF16),
        "b2s": np.asarray(inputs["b2s"], dtype=_F32).reshape(L, F, 1),
    }

    phys = struct["phys"]
    in_maps = []
    for c in range(NC):
        xs = np.zeros((F, PERP), dtype=_F32)
        xs[:, :PER] = x[c * PER:(c + 1) * PER].T
        xs = xs.reshape(F, NBLK, BLK)[:, phys[c], :].reshape(F, PERP)
        m = dict(shared)
        m["xT"] = np.ascontiguousarray(xs.astype(_BF16))
        m["idx"] = struct["idx_planes"][c]
        m["dstrel"] = struct["dstrel_planes"][c]
        m["ohg"] = struct["ohg_planes"][c]
        in_maps.append(m)
    return in_maps


def kernel(**inputs):
    from concourse.bass_utils import run_bass_kernel_spmd

    edge_index = np.asarray(inputs["edge_index"])
    batch = np.asarray(inputs["batch"])
    eps = np.asarray(inputs["eps"], dtype=_F32)

    struct = _build_structures(edge_index, batch)
    nc = _build_program(struct, [float(e) for e in eps])
    in_maps = _make_in_maps(struct, inputs)

    res = run_bass_kernel_spmd(nc, in_maps, core_ids=list(range(NC)))
    out = np.zeros((G, L * F), dtype=_F32)
    for c in range(NC):
        out += res.results[c]["out"]
    return out
